# revision 7
# baseline (speedup 1.0000x reference)
"""Continuous positional bias kernel for Trainium2 (8 NeuronCores).

Reference computation (per batch b):
    rel[q,k,:] = query_coords[b,q,:] - key_coords[b,k,:]        (2 coords)
    h1 = relu(rel @ w1 + b1)      # (Nq,Nk,128)
    h2 = relu(h1 @ w2 + b2)       # (Nq,Nk,128)
    out[b,:,q,k] = (h2 @ w3 + b3).T  # (heads=8, Nq, Nk)

Layer 1 is linear in rel = q - k, so
    w1^T rel + b1 = (w1^T q + b1) + (-w1^T k) = beta[:,q] + gamma[:,k]
with beta/gamma computed on host.  Layer 3 contracts 128 hidden dims to
just 8 heads (6% of the FLOPs) — it is ALSO computed on host, from h2
shipped back in bf16.  Rationale (from the v1 trace): on-device L3 cost
~95us of PE time (each [8,128,512] matmul streams 512 columns, the same
cost as a [128,128,512] one) plus ~89us of DVE/ACT time for the
PSUM->SBUF copies of its output — while all three engines sat at
81-92% busy.  Dropping L3 removes both, leaving:

    h1 = relu(gamma + beta_col)     DVE tensor_scalar, bf16 src (~478ns)
    h2 = relu(w2^T h1 + b2)         PE matmul into PSUM pair-tiles;
                                    relu drain [128,2048] split DVE/ACT
    h2 -> HBM                       4MB chunked DMAs (~400GB/s)

Drains run on 2-query PSUM tiles ([128,2048] f32 = 4 banks, bufs=2 =
all 8 banks) to amortize the fixed op overhead: ACT (N+352)/1.2 =
2000ns/pair, DVE (120+N)/0.96 = 2258ns/pair.  Schedule: DVE takes every
4th pair-drain plus all h1s; ACT the rest.  Per-query steady state
~760ns vs ~960ns for v1.

b1 is folded into beta; b2/b3 are zeros in this problem's generator
(kernel() asserts and falls back to a bias-carrying drain if not).

Sharding: 8 cores x (batch, 256 queries). Weights replicated.
"""

import numpy as np

B, NQ, NK, H, HD = 2, 1024, 1024, 8, 128
NCORES = 8
CPB = NCORES // B          # cores per batch = 4
QPC = NQ // CPB            # queries per core = 256
KT = 512                   # matmul moving free dim (one PSUM bank)
CHQ = 16                   # queries per output DMA chunk (4MB bf16)

_CACHE = {}


def _build_nc(with_b2: bool):
    from contextlib import ExitStack

    import concourse.bass as bass
    import concourse.tile as tile
    from concourse import bacc, mybir
    from concourse.alu_op_type import AluOpType

    f32 = mybir.dt.float32
    bf16 = mybir.dt.bfloat16
    Relu = mybir.ActivationFunctionType.Relu

    nc = bacc.Bacc(
        "TRN2",
        target_bir_lowering=False,
        debug=False,
        enable_asserts=True,
        num_devices=NCORES,
    )

    gamma_d = nc.dram_tensor("gamma", (HD, NK), bf16, kind="ExternalInput").ap()
    beta_d = nc.dram_tensor("beta", (HD, QPC), f32, kind="ExternalInput").ap()
    w2_d = nc.dram_tensor("w2", (HD, HD), bf16, kind="ExternalInput").ap()
    if with_b2:
        b2_d = nc.dram_tensor("b2", (HD, 1), f32, kind="ExternalInput").ap()
    out_d = nc.dram_tensor("out", (HD, QPC, NK), bf16, kind="ExternalOutput").ap()

    npairs = QPC // 2
    ppc = CHQ // 2             # pairs per chunk = 8
    nchunks = QPC // CHQ

    with tile.TileContext(nc) as tc:
        with ExitStack() as ctx:
            consts = ctx.enter_context(tc.tile_pool(name="consts", bufs=1))
            h1p = ctx.enter_context(tc.tile_pool(name="h1p", bufs=6))
            h2p = ctx.enter_context(tc.tile_pool(name="h2p", bufs=4))
            ps2 = ctx.enter_context(tc.tile_pool(name="ps2", bufs=2, space="PSUM"))

            # input DMAs fan out across engine queues (serialized on one
            # queue they cost ~2us each in semaphore overhead)
            gamma_b = consts.tile([HD, NK], bf16)
            nc.sync.dma_start(gamma_b, gamma_d)
            beta = consts.tile([HD, QPC], f32)
            nc.scalar.dma_start(beta, beta_d)
            w2r = consts.tile([HD, HD], bf16)
            nc.scalar.dma_start(w2r, w2_d)
            if with_b2:
                b2 = consts.tile([HD, 1], f32)
                nc.scalar.dma_start(b2, b2_d)

            def make_h1(q):
                h1 = h1p.tile([HD, NK], bf16, tag="h1")
                nc.vector.tensor_scalar(
                    h1, gamma_b, beta[:, q:q + 1], 0.0,
                    AluOpType.add, AluOpType.max,
                )
                return h1

            h1t = {}
            chunks = {}

            H1_AHEAD = 4
            for q in range(H1_AHEAD):
                h1t[q] = make_h1(q)

            for p in range(npairs):
                c, cp = p // ppc, p % ppc       # chunk id, pair-in-chunk
                if cp == 0:
                    h2c = h2p.tile([HD, CHQ * NK], bf16, tag="h2c")
                    chunks[c] = h2c

                # next h1s first: their input (gamma) is always ready, so
                # they never block the DVE queue; the PE needs them soon.
                for j in range(2):
                    qa = 2 * p + j + H1_AHEAD
                    if qa < QPC:
                        h1t[qa] = make_h1(qa)

                p2 = ps2.tile([HD, 2 * NK], f32, tag="p2")
                for j in range(2):
                    h1 = h1t.pop(2 * p + j)
                    for kh in range(2):
                        nc.tensor.matmul(
                            p2[:, j * NK + kh * KT:j * NK + (kh + 1) * KT],
                            w2r,
                            h1[:, kh * KT:(kh + 1) * KT],
                            start=True,
                            stop=True,
                        )

                dst = chunks[c][:, 2 * cp * NK:(2 * cp + 2) * NK]
                # Both engines drain every pair concurrently on disjoint
                # column ranges (different PSUM banks): ACT is 1 elem/cy
                # @1.2GHz with a 310cy overhead, DVE 1 elem/cy @0.96GHz
                # with 120cy, and DVE also carries the h1s.  D=1600/448
                # balances DVE (2*481 + ~640) vs ACT (~1590) per pair and,
                # unlike alternating whole drains between engines, leaves
                # no engine idle waiting for the other's tile to refill.
                DS = 2 * NK - 448
                if with_b2:
                    nc.scalar.activation(dst[:, :DS], p2[:, :DS], Relu, bias=b2)
                    nc.vector.tensor_scalar(
                        dst[:, DS:], p2[:, DS:], b2, 0.0,
                        AluOpType.add, AluOpType.max,
                    )
                else:
                    nc.scalar.activation(dst[:, :DS], p2[:, :DS], Relu)
                    nc.vector.tensor_scalar_max(dst[:, DS:], p2[:, DS:], 0.0)

                # chunk complete -> one big contiguous DMA (q,k fused dim).
                # Final chunk goes out in two halves so the second half's
                # store overlaps nothing but the pipeline tail.
                last = c == nchunks - 1
                if last and cp == ppc // 2 - 1:
                    half = bass.AP(
                        tensor=out_d.tensor,
                        offset=out_d.offset + c * CHQ * NK,
                        ap=[[QPC * NK, HD], [1, CHQ * NK // 2]],
                    )
                    nc.sync.dma_start(half, chunks[c][:, :CHQ * NK // 2])
                elif cp == ppc - 1:
                    tile_src = chunks.pop(c)
                    if last:
                        half = bass.AP(
                            tensor=out_d.tensor,
                            offset=out_d.offset + c * CHQ * NK + CHQ * NK // 2,
                            ap=[[QPC * NK, HD], [1, CHQ * NK // 2]],
                        )
                        nc.sync.dma_start(half, tile_src[:, CHQ * NK // 2:])
                    else:
                        dest = bass.AP(
                            tensor=out_d.tensor,
                            offset=out_d.offset + c * CHQ * NK,
                            ap=[[QPC * NK, HD], [1, CHQ * NK]],
                        )
                        nc.sync.dma_start(dest, tile_src)

    nc.compile()
    return nc


def _get_nc(with_b2: bool):
    key = ("nc", with_b2)
    if key not in _CACHE:
        _CACHE[key] = _build_nc(with_b2)
    return _CACHE[key]


def make_in_maps(query_coords, key_coords, w1, b1, w2, b2):
    """Host-side shard prep: per-core gamma/beta + replicated weights."""
    qc = np.asarray(query_coords, np.float32)
    kc = np.asarray(key_coords, np.float32)
    w1 = np.asarray(w1, np.float32)
    b1 = np.asarray(b1, np.float32)
    w2 = np.asarray(w2, np.float32)
    b2 = np.asarray(b2, np.float32)

    import ml_dtypes

    with_b2 = bool(np.any(b2))
    w2c = np.ascontiguousarray(w2.astype(ml_dtypes.bfloat16))
    b2c = np.ascontiguousarray(b2.reshape(HD, 1))

    in_maps = []
    for c in range(NCORES):
        b = c // CPB
        q0 = (c % CPB) * QPC
        gamma = np.ascontiguousarray(
            (-(kc[b] @ w1).T).astype(ml_dtypes.bfloat16)         # (128, NK)
        )
        beta = np.ascontiguousarray(
            (qc[b, q0:q0 + QPC] @ w1).T + b1[:, None]            # (128, QPC)
        )
        m = {"gamma": gamma, "beta": beta, "w2": w2c}
        if with_b2:
            m["b2"] = b2c
        in_maps.append(m)
    return in_maps, with_b2


def assemble_output(results, w3, b3):
    """Host layer 3: gather per-core h2 [HD, QPC, NK] bf16, contract the
    128 hidden dims to 8 heads in f32, into (B, H, NQ, NK) f32."""
    w3 = np.asarray(w3, np.float32)
    b3 = np.asarray(b3, np.float32)
    out = np.empty((B, H, NQ, NK), np.float32)
    w3t = np.ascontiguousarray(w3.T)                             # (H, HD)
    for c in range(NCORES):
        b = c // CPB
        q0 = (c % CPB) * QPC
        h2 = results[c]["out"]
        # exact bf16 -> f32 without ml_dtypes' slow cast path
        h2f = (
            (h2.view(np.uint16).astype(np.uint32) << 16)
            .view(np.float32)
            .reshape(HD, QPC * NK)
        )
        out[b, :, q0:q0 + QPC, :] = (w3t @ h2f).reshape(H, QPC, NK)
    if np.any(b3):
        out += b3.reshape(1, H, 1, 1)
    return out


def kernel(**inputs):
    from concourse.bass_utils import run_bass_kernel_spmd

    in_maps, with_b2 = make_in_maps(
        inputs["query_coords"],
        inputs["key_coords"],
        inputs["w1"],
        inputs["b1"],
        inputs["w2"],
        inputs["b2"],
    )
    nc = _get_nc(with_b2)
    res = run_bass_kernel_spmd(nc, in_maps, list(range(NCORES)))
    return assemble_output(res.results, inputs["w3"], inputs["b3"])


# revision 8
# speedup vs baseline: 1.5354x; 1.5354x over previous
"""Continuous positional bias kernel for Trainium2 (8 NeuronCores).

Reference computation (per batch b):
    rel[q,k,:] = query_coords[b,q,:] - key_coords[b,k,:]        (2 coords)
    h1 = relu(rel @ w1 + b1)      # (Nq,Nk,128)
    h2 = relu(h1 @ w2 + b2)       # (Nq,Nk,128)
    out[b,:,q,k] = (h2 @ w3 + b3).T  # (heads=8, Nq, Nk)

Layer 1 is linear in rel = q - k, so
    w1^T rel + b1 = (w1^T q + b1) + (-w1^T k) = beta[:,q] + gamma[:,k]
with beta/gamma computed on host.  Layer 3 contracts 128 hidden dims to
just 8 heads (6% of the FLOPs) — it is ALSO computed on host, from h2
shipped back in bf16.  Rationale (from the v1 trace): on-device L3 cost
~95us of PE time (each [8,128,512] matmul streams 512 columns, the same
cost as a [128,128,512] one) plus ~89us of DVE/ACT time for the
PSUM->SBUF copies of its output — while all three engines sat at
81-92% busy.  Dropping L3 removes both, leaving:

    h1 = relu(gamma + beta_col)     DVE tensor_scalar, bf16 src (~478ns)
    h2 = relu(w2^T h1 + b2)         PE matmul into PSUM pair-tiles;
                                    relu drain [128,2048] split DVE/ACT
    h2 -> HBM                       4MB chunked DMAs (~400GB/s)

Drains run on 2-query PSUM tiles ([128,2048] f32 = 4 banks, bufs=2 =
all 8 banks) to amortize the fixed op overhead: ACT (N+352)/1.2 =
2000ns/pair, DVE (120+N)/0.96 = 2258ns/pair.  Schedule: DVE takes every
4th pair-drain plus all h1s; ACT the rest.  Per-query steady state
~760ns vs ~960ns for v1.

b1 is folded into beta; b2/b3 are zeros in this problem's generator
(kernel() asserts and falls back to a bias-carrying drain if not).

Sharding: 8 cores x (batch, 256 queries). Weights replicated.
"""

import numpy as np

B, NQ, NK, H, HD = 2, 1024, 1024, 8, 128
NCORES = 8
CPB = NCORES // B          # cores per batch = 4
QPC = NQ // CPB            # queries per core = 256
KT = 512                   # matmul moving free dim (one PSUM bank)
CHQ = 16                   # queries per output DMA chunk (4MB bf16)

_CACHE = {}


def _build_nc(with_b2: bool):
    from contextlib import ExitStack

    import concourse.bass as bass
    import concourse.tile as tile
    from concourse import bacc, mybir
    from concourse.alu_op_type import AluOpType

    f32 = mybir.dt.float32
    bf16 = mybir.dt.bfloat16
    Relu = mybir.ActivationFunctionType.Relu

    nc = bacc.Bacc(
        "TRN2",
        target_bir_lowering=False,
        debug=False,
        enable_asserts=True,
        num_devices=NCORES,
    )

    gamma_d = nc.dram_tensor("gamma", (HD, NK), bf16, kind="ExternalInput").ap()
    beta_d = nc.dram_tensor("beta", (HD, QPC), f32, kind="ExternalInput").ap()
    w2_d = nc.dram_tensor("w2", (HD, HD), bf16, kind="ExternalInput").ap()
    if with_b2:
        b2_d = nc.dram_tensor("b2", (HD, 1), f32, kind="ExternalInput").ap()
    out_d = nc.dram_tensor("out", (HD, QPC, NK), bf16, kind="ExternalOutput").ap()

    npairs = QPC // 2
    ppc = CHQ // 2             # pairs per chunk = 8
    nchunks = QPC // CHQ

    with tile.TileContext(nc) as tc:
        with ExitStack() as ctx:
            consts = ctx.enter_context(tc.tile_pool(name="consts", bufs=1))
            h1p = ctx.enter_context(tc.tile_pool(name="h1p", bufs=6))
            h2p = ctx.enter_context(tc.tile_pool(name="h2p", bufs=4))
            ps2 = ctx.enter_context(tc.tile_pool(name="ps2", bufs=2, space="PSUM"))

            # input DMAs fan out across engine queues (serialized on one
            # queue they cost ~2us each in semaphore overhead)
            gamma_b = consts.tile([HD, NK], bf16)
            nc.sync.dma_start(gamma_b, gamma_d)
            beta = consts.tile([HD, QPC], f32)
            nc.scalar.dma_start(beta, beta_d)
            w2r = consts.tile([HD, HD], bf16)
            nc.scalar.dma_start(w2r, w2_d)
            if with_b2:
                b2 = consts.tile([HD, 1], f32)
                nc.scalar.dma_start(b2, b2_d)

            def make_h1(q):
                h1 = h1p.tile([HD, NK], bf16, tag="h1")
                nc.vector.tensor_scalar(
                    h1, gamma_b, beta[:, q:q + 1], 0.0,
                    AluOpType.add, AluOpType.max,
                )
                return h1

            h1t = {}
            chunks = {}

            H1_AHEAD = 4
            for q in range(H1_AHEAD):
                h1t[q] = make_h1(q)

            for p in range(npairs):
                c, cp = p // ppc, p % ppc       # chunk id, pair-in-chunk
                if cp == 0:
                    h2c = h2p.tile([HD, CHQ * NK], bf16, tag="h2c")
                    chunks[c] = h2c

                # next h1s first: their input (gamma) is always ready, so
                # they never block the DVE queue; the PE needs them soon.
                for j in range(2):
                    qa = 2 * p + j + H1_AHEAD
                    if qa < QPC:
                        h1t[qa] = make_h1(qa)

                p2 = ps2.tile([HD, 2 * NK], f32, tag="p2")
                for j in range(2):
                    h1 = h1t.pop(2 * p + j)
                    for kh in range(2):
                        nc.tensor.matmul(
                            p2[:, j * NK + kh * KT:j * NK + (kh + 1) * KT],
                            w2r,
                            h1[:, kh * KT:(kh + 1) * KT],
                            start=True,
                            stop=True,
                        )

                dst = chunks[c][:, 2 * cp * NK:(2 * cp + 2) * NK]
                # Both engines drain every pair concurrently on disjoint
                # column ranges (different PSUM banks): ACT is 1 elem/cy
                # @1.2GHz with a 310cy overhead, DVE 1 elem/cy @0.96GHz
                # with 120cy, and DVE also carries the h1s.  D=1600/448
                # balances DVE (2*481 + ~640) vs ACT (~1590) per pair and,
                # unlike alternating whole drains between engines, leaves
                # no engine idle waiting for the other's tile to refill.
                DS = 2 * NK - 512    # bank-aligned: ACT banks 0-2, DVE bank 3
                if with_b2:
                    nc.scalar.activation(dst[:, :DS], p2[:, :DS], Relu, bias=b2)
                    nc.vector.tensor_scalar(
                        dst[:, DS:], p2[:, DS:], b2, 0.0,
                        AluOpType.add, AluOpType.max,
                    )
                else:
                    nc.scalar.activation(dst[:, :DS], p2[:, :DS], Relu)
                    nc.vector.tensor_scalar_max(dst[:, DS:], p2[:, DS:], 0.0)

                # chunk complete -> one big contiguous DMA (q,k fused dim).
                # Final chunk goes out in two halves so the second half's
                # store overlaps nothing but the pipeline tail.
                last = c == nchunks - 1
                if last and cp == ppc // 2 - 1:
                    half = bass.AP(
                        tensor=out_d.tensor,
                        offset=out_d.offset + c * CHQ * NK,
                        ap=[[QPC * NK, HD], [1, CHQ * NK // 2]],
                    )
                    nc.sync.dma_start(half, chunks[c][:, :CHQ * NK // 2])
                elif cp == ppc - 1:
                    tile_src = chunks.pop(c)
                    if last:
                        half = bass.AP(
                            tensor=out_d.tensor,
                            offset=out_d.offset + c * CHQ * NK + CHQ * NK // 2,
                            ap=[[QPC * NK, HD], [1, CHQ * NK // 2]],
                        )
                        nc.sync.dma_start(half, tile_src[:, CHQ * NK // 2:])
                    else:
                        dest = bass.AP(
                            tensor=out_d.tensor,
                            offset=out_d.offset + c * CHQ * NK,
                            ap=[[QPC * NK, HD], [1, CHQ * NK]],
                        )
                        nc.sync.dma_start(dest, tile_src)

    nc.compile()
    return nc


def _get_nc(with_b2: bool):
    key = ("nc", with_b2)
    if key not in _CACHE:
        _CACHE[key] = _build_nc(with_b2)
    return _CACHE[key]


def make_in_maps(query_coords, key_coords, w1, b1, w2, b2):
    """Host-side shard prep: per-core gamma/beta + replicated weights."""
    qc = np.asarray(query_coords, np.float32)
    kc = np.asarray(key_coords, np.float32)
    w1 = np.asarray(w1, np.float32)
    b1 = np.asarray(b1, np.float32)
    w2 = np.asarray(w2, np.float32)
    b2 = np.asarray(b2, np.float32)

    import ml_dtypes

    with_b2 = bool(np.any(b2))
    w2c = np.ascontiguousarray(w2.astype(ml_dtypes.bfloat16))
    b2c = np.ascontiguousarray(b2.reshape(HD, 1))

    in_maps = []
    for c in range(NCORES):
        b = c // CPB
        q0 = (c % CPB) * QPC
        gamma = np.ascontiguousarray(
            (-(kc[b] @ w1).T).astype(ml_dtypes.bfloat16)         # (128, NK)
        )
        beta = np.ascontiguousarray(
            (qc[b, q0:q0 + QPC] @ w1).T + b1[:, None]            # (128, QPC)
        )
        m = {"gamma": gamma, "beta": beta, "w2": w2c}
        if with_b2:
            m["b2"] = b2c
        in_maps.append(m)
    return in_maps, with_b2


def assemble_output(results, w3, b3):
    """Host layer 3: gather per-core h2 [HD, QPC, NK] bf16, contract the
    128 hidden dims to 8 heads in f32, into (B, H, NQ, NK) f32."""
    w3 = np.asarray(w3, np.float32)
    b3 = np.asarray(b3, np.float32)
    out = np.empty((B, H, NQ, NK), np.float32)
    w3t = np.ascontiguousarray(w3.T)                             # (H, HD)
    for c in range(NCORES):
        b = c // CPB
        q0 = (c % CPB) * QPC
        h2 = results[c]["out"]
        # exact bf16 -> f32 without ml_dtypes' slow cast path
        h2f = (
            (h2.view(np.uint16).astype(np.uint32) << 16)
            .view(np.float32)
            .reshape(HD, QPC * NK)
        )
        out[b, :, q0:q0 + QPC, :] = (w3t @ h2f).reshape(H, QPC, NK)
    if np.any(b3):
        out += b3.reshape(1, H, 1, 1)
    return out


def kernel(**inputs):
    from concourse.bass_utils import run_bass_kernel_spmd

    in_maps, with_b2 = make_in_maps(
        inputs["query_coords"],
        inputs["key_coords"],
        inputs["w1"],
        inputs["b1"],
        inputs["w2"],
        inputs["b2"],
    )
    nc = _get_nc(with_b2)
    res = run_bass_kernel_spmd(nc, in_maps, list(range(NCORES)))
    return assemble_output(res.results, inputs["w3"], inputs["b3"])


# revision 11
# speedup vs baseline: 1.8691x; 1.2173x over previous
"""Continuous positional bias kernel for Trainium2 (8 NeuronCores).

Reference computation (per batch b):
    rel[q,k,:] = query_coords[b,q,:] - key_coords[b,k,:]        (2 coords)
    h1 = relu(rel @ w1 + b1)      # (Nq,Nk,128)
    h2 = relu(h1 @ w2 + b2)       # (Nq,Nk,128)
    out[b,:,q,k] = (h2 @ w3 + b3).T  # (heads=8, Nq, Nk)

Layer 1 is linear in rel = q - k, so
    w1^T rel + b1 = (w1^T q + b1) + (-w1^T k) = beta[:,q] + gamma[:,k]
with beta/gamma computed on host.  Layer 3 contracts 128 hidden dims to
just 8 heads (6% of the FLOPs) — it is ALSO computed on host, from h2
shipped back in bf16.  Rationale (from the v1 trace): on-device L3 cost
~95us of PE time (each [8,128,512] matmul streams 512 columns, the same
cost as a [128,128,512] one) plus ~89us of DVE/ACT time for the
PSUM->SBUF copies of its output — while all three engines sat at
81-92% busy.  Dropping L3 removes both, leaving:

    h1 = relu(gamma + beta_col)     DVE tensor_scalar, bf16 src (~478ns)
    h2 = relu(w2^T h1 + b2)         PE matmul into PSUM pair-tiles;
                                    relu drain [128,2048] split DVE/ACT
    h2 -> HBM                       4MB chunked DMAs (~400GB/s)

Drains run on 2-query PSUM tiles ([128,2048] f32 = 4 banks, bufs=2 =
all 8 banks) to amortize the fixed op overhead: ACT (N+352)/1.2 =
2000ns/pair, DVE (120+N)/0.96 = 2258ns/pair.  Schedule: DVE takes every
4th pair-drain plus all h1s; ACT the rest.  Per-query steady state
~760ns vs ~960ns for v1.

b1 is folded into beta; b2/b3 are zeros in this problem's generator
(kernel() asserts and falls back to a bias-carrying drain if not).

Sharding: 8 cores x (batch, 256 queries). Weights replicated.
"""

import numpy as np

B, NQ, NK, H, HD = 2, 1024, 1024, 8, 128
NCORES = 8
CPB = NCORES // B          # cores per batch = 4
QPC = NQ // CPB            # queries per core = 256
KT = 512                   # matmul moving free dim (one PSUM bank)
CHQ = 16                   # queries per output DMA chunk (4MB bf16)

_CACHE = {}


def _build_nc(with_b2: bool):
    from contextlib import ExitStack

    import concourse.bass as bass
    import concourse.tile as tile
    from concourse import bacc, mybir
    from concourse.alu_op_type import AluOpType

    f32 = mybir.dt.float32
    bf16 = mybir.dt.bfloat16
    Relu = mybir.ActivationFunctionType.Relu

    nc = bacc.Bacc(
        "TRN2",
        target_bir_lowering=False,
        debug=False,
        enable_asserts=True,
        num_devices=NCORES,
    )

    gamma_d = nc.dram_tensor("gamma", (HD, NK), bf16, kind="ExternalInput").ap()
    beta_d = nc.dram_tensor("beta", (HD, QPC), f32, kind="ExternalInput").ap()
    w2_d = nc.dram_tensor("w2", (HD, HD), bf16, kind="ExternalInput").ap()
    if with_b2:
        b2_d = nc.dram_tensor("b2", (HD, 1), f32, kind="ExternalInput").ap()
    out_d = nc.dram_tensor("out", (HD, QPC, NK), bf16, kind="ExternalOutput").ap()

    nchunks = QPC // CHQ

    with tile.TileContext(nc) as tc:
        with ExitStack() as ctx:
            consts = ctx.enter_context(tc.tile_pool(name="consts", bufs=1))
            h1p = ctx.enter_context(tc.tile_pool(name="h1p", bufs=8))
            h2p = ctx.enter_context(tc.tile_pool(name="h2p", bufs=4))
            ps2 = ctx.enter_context(tc.tile_pool(name="ps2", bufs=4, space="PSUM"))

            # input DMAs fan out across engine queues (serialized on one
            # queue they cost ~2us each in semaphore overhead)
            gamma_b = consts.tile([HD, NK], bf16)
            nc.sync.dma_start(gamma_b, gamma_d)
            beta = consts.tile([HD, QPC], f32)
            nc.scalar.dma_start(beta, beta_d)
            w2r = consts.tile([HD, HD], bf16)
            nc.scalar.dma_start(w2r, w2_d)
            if with_b2:
                b2 = consts.tile([HD, 1], f32)
                nc.scalar.dma_start(b2, b2_d)

            def make_h1(q):
                h1 = h1p.tile([HD, NK], bf16, tag="h1")
                nc.vector.tensor_scalar(
                    h1, gamma_b, beta[:, q:q + 1], 0.0,
                    AluOpType.add, AluOpType.max,
                )
                return h1

            h1t = {}
            chunks = {}

            # 70 of 256 drains go to DVE (it also carries all h1s), the
            # rest to ACT: balances DVE 481+1240f vs ACT 1135(1-f).
            NDVE = 70
            H1_AHEAD = 5
            for q in range(H1_AHEAD):
                h1t[q] = make_h1(q)

            for q in range(QPC):
                c, pos = q // CHQ, q % CHQ
                if pos == 0:
                    h2c = h2p.tile([HD, CHQ * NK], bf16, tag="h2c")
                    chunks[c] = h2c

                # next h1 first: its input (gamma) is always ready, so it
                # never blocks the DVE queue; the PE needs it soon.
                if q + H1_AHEAD < QPC:
                    h1t[q + H1_AHEAD] = make_h1(q + H1_AHEAD)

                p2 = ps2.tile([HD, NK], f32, tag="p2")
                h1 = h1t.pop(q)
                for kh in range(2):
                    nc.tensor.matmul(
                        p2[:, kh * KT:(kh + 1) * KT],
                        w2r,
                        h1[:, kh * KT:(kh + 1) * KT],
                        start=True,
                        stop=True,
                    )

                dst = chunks[c][:, pos * NK:(pos + 1) * NK]
                dve_drain = (q * NDVE) // QPC != ((q + 1) * NDVE) // QPC
                if dve_drain:
                    if with_b2:
                        nc.vector.tensor_scalar(
                            dst, p2, b2, 0.0, AluOpType.add, AluOpType.max,
                        )
                    else:
                        nc.vector.tensor_scalar_max(dst, p2, 0.0)
                else:
                    if with_b2:
                        nc.scalar.activation(dst, p2, Relu, bias=b2)
                    else:
                        nc.scalar.activation(dst, p2, Relu)

                # chunk complete -> one big contiguous DMA (q,k fused dim).
                # Final chunk goes out in two halves so the second half's
                # store overlaps nothing but the pipeline tail.
                last = c == nchunks - 1
                if last and pos == CHQ // 2 - 1:
                    half = bass.AP(
                        tensor=out_d.tensor,
                        offset=out_d.offset + c * CHQ * NK,
                        ap=[[QPC * NK, HD], [1, CHQ * NK // 2]],
                    )
                    nc.sync.dma_start(half, chunks[c][:, :CHQ * NK // 2])
                elif pos == CHQ - 1:
                    tile_src = chunks.pop(c)
                    if last:
                        half = bass.AP(
                            tensor=out_d.tensor,
                            offset=out_d.offset + c * CHQ * NK + CHQ * NK // 2,
                            ap=[[QPC * NK, HD], [1, CHQ * NK // 2]],
                        )
                        nc.sync.dma_start(half, tile_src[:, CHQ * NK // 2:])
                    else:
                        dest = bass.AP(
                            tensor=out_d.tensor,
                            offset=out_d.offset + c * CHQ * NK,
                            ap=[[QPC * NK, HD], [1, CHQ * NK]],
                        )
                        nc.sync.dma_start(dest, tile_src)

    nc.compile()
    return nc


def _get_nc(with_b2: bool):
    key = ("nc", with_b2)
    if key not in _CACHE:
        _CACHE[key] = _build_nc(with_b2)
    return _CACHE[key]


def make_in_maps(query_coords, key_coords, w1, b1, w2, b2):
    """Host-side shard prep: per-core gamma/beta + replicated weights."""
    qc = np.asarray(query_coords, np.float32)
    kc = np.asarray(key_coords, np.float32)
    w1 = np.asarray(w1, np.float32)
    b1 = np.asarray(b1, np.float32)
    w2 = np.asarray(w2, np.float32)
    b2 = np.asarray(b2, np.float32)

    import ml_dtypes

    with_b2 = bool(np.any(b2))
    w2c = np.ascontiguousarray(w2.astype(ml_dtypes.bfloat16))
    b2c = np.ascontiguousarray(b2.reshape(HD, 1))

    in_maps = []
    for c in range(NCORES):
        b = c // CPB
        q0 = (c % CPB) * QPC
        gamma = np.ascontiguousarray(
            (-(kc[b] @ w1).T).astype(ml_dtypes.bfloat16)         # (128, NK)
        )
        beta = np.ascontiguousarray(
            (qc[b, q0:q0 + QPC] @ w1).T + b1[:, None]            # (128, QPC)
        )
        m = {"gamma": gamma, "beta": beta, "w2": w2c}
        if with_b2:
            m["b2"] = b2c
        in_maps.append(m)
    return in_maps, with_b2


def assemble_output(results, w3, b3):
    """Host layer 3: gather per-core h2 [HD, QPC, NK] bf16, contract the
    128 hidden dims to 8 heads in f32, into (B, H, NQ, NK) f32."""
    w3 = np.asarray(w3, np.float32)
    b3 = np.asarray(b3, np.float32)
    out = np.empty((B, H, NQ, NK), np.float32)
    w3t = np.ascontiguousarray(w3.T)                             # (H, HD)
    for c in range(NCORES):
        b = c // CPB
        q0 = (c % CPB) * QPC
        h2 = results[c]["out"]
        # exact bf16 -> f32 without ml_dtypes' slow cast path
        h2f = (
            (h2.view(np.uint16).astype(np.uint32) << 16)
            .view(np.float32)
            .reshape(HD, QPC * NK)
        )
        out[b, :, q0:q0 + QPC, :] = (w3t @ h2f).reshape(H, QPC, NK)
    if np.any(b3):
        out += b3.reshape(1, H, 1, 1)
    return out


def kernel(**inputs):
    from concourse.bass_utils import run_bass_kernel_spmd

    in_maps, with_b2 = make_in_maps(
        inputs["query_coords"],
        inputs["key_coords"],
        inputs["w1"],
        inputs["b1"],
        inputs["w2"],
        inputs["b2"],
    )
    nc = _get_nc(with_b2)
    res = run_bass_kernel_spmd(nc, in_maps, list(range(NCORES)))
    return assemble_output(res.results, inputs["w3"], inputs["b3"])


# revision 14
# speedup vs baseline: 1.9139x; 1.0240x over previous
"""Continuous positional bias kernel for Trainium2 (8 NeuronCores).

Reference computation (per batch b):
    rel[q,k,:] = query_coords[b,q,:] - key_coords[b,k,:]        (2 coords)
    h1 = relu(rel @ w1 + b1)      # (Nq,Nk,128)
    h2 = relu(h1 @ w2 + b2)       # (Nq,Nk,128)
    out[b,:,q,k] = (h2 @ w3 + b3).T  # (heads=8, Nq, Nk)

Layer 1 is linear in rel = q - k, so
    w1^T rel + b1 = (w1^T q + b1) + (-w1^T k) = beta[:,q] + gamma[:,k]
with beta/gamma computed on host.  Layer 3 contracts 128 hidden dims to
just 8 heads (6% of the FLOPs) — it is ALSO computed on host, from h2
shipped back in bf16.  Rationale (from the v1 trace): on-device L3 cost
~95us of PE time (each [8,128,512] matmul streams 512 columns, the same
cost as a [128,128,512] one) plus ~89us of DVE/ACT time for the
PSUM->SBUF copies of its output — while all three engines sat at
81-92% busy.  Dropping L3 removes both, leaving:

    h1 = relu(gamma + beta_col)     DVE tensor_scalar, bf16 src (~478ns)
    h2 = relu(w2^T h1 + b2)         PE matmul into PSUM pair-tiles;
                                    relu drain [128,2048] split DVE/ACT
    h2 -> HBM                       4MB chunked DMAs (~400GB/s)

Drains run on 2-query PSUM tiles ([128,2048] f32 = 4 banks, bufs=2 =
all 8 banks) to amortize the fixed op overhead: ACT (N+352)/1.2 =
2000ns/pair, DVE (120+N)/0.96 = 2258ns/pair.  Schedule: DVE takes every
4th pair-drain plus all h1s; ACT the rest.  Per-query steady state
~760ns vs ~960ns for v1.

b1 is folded into beta; b2/b3 are zeros in this problem's generator
(kernel() asserts and falls back to a bias-carrying drain if not).

Sharding: 8 cores x (batch, 256 queries). Weights replicated.
"""

import numpy as np

B, NQ, NK, H, HD = 2, 1024, 1024, 8, 128
NCORES = 8
CPB = NCORES // B          # cores per batch = 4
QPC = NQ // CPB            # queries per core = 256
KT = 512                   # matmul moving free dim (one PSUM bank)
CHQ = 16                   # queries per output DMA chunk (4MB bf16)

_CACHE = {}


def _build_nc(with_b2: bool):
    from contextlib import ExitStack

    import concourse.bass as bass
    import concourse.tile as tile
    from concourse import bacc, mybir
    from concourse.alu_op_type import AluOpType

    f32 = mybir.dt.float32
    bf16 = mybir.dt.bfloat16
    Relu = mybir.ActivationFunctionType.Relu

    nc = bacc.Bacc(
        "TRN2",
        target_bir_lowering=False,
        debug=False,
        enable_asserts=True,
        num_devices=NCORES,
    )

    gamma_d = nc.dram_tensor("gamma", (HD, NK), bf16, kind="ExternalInput").ap()
    beta_d = nc.dram_tensor("beta", (HD, QPC), f32, kind="ExternalInput").ap()
    w2_d = nc.dram_tensor("w2", (HD, HD), bf16, kind="ExternalInput").ap()
    if with_b2:
        b2_d = nc.dram_tensor("b2", (HD, 1), f32, kind="ExternalInput").ap()
    out_d = nc.dram_tensor("out", (HD, QPC, NK), bf16, kind="ExternalOutput").ap()

    nchunks = QPC // CHQ

    with tile.TileContext(nc) as tc:
        with ExitStack() as ctx:
            consts = ctx.enter_context(tc.tile_pool(name="consts", bufs=1))
            h1p = ctx.enter_context(tc.tile_pool(name="h1p", bufs=8))
            h2p = ctx.enter_context(tc.tile_pool(name="h2p", bufs=4))
            ps2 = ctx.enter_context(tc.tile_pool(name="ps2", bufs=4, space="PSUM"))

            # input DMAs fan out across engine queues (serialized on one
            # queue they cost ~2us each in semaphore overhead)
            gamma_b = consts.tile([HD, NK], bf16)
            nc.sync.dma_start(gamma_b, gamma_d)
            beta = consts.tile([HD, QPC], f32)
            nc.scalar.dma_start(beta, beta_d)
            w2r = consts.tile([HD, HD], bf16)
            nc.scalar.dma_start(w2r, w2_d)
            if with_b2:
                b2 = consts.tile([HD, 1], f32)
                nc.scalar.dma_start(b2, b2_d)

            # touch the Relu table at t~0 so the one-time ACT_TABLE_LOAD
            # (~2.7us) overlaps the input DMAs instead of delaying the
            # first real drain
            warm = consts.tile([HD, 1], f32)
            nc.vector.memset(warm, 0.0)
            nc.scalar.activation(warm, warm, Relu)

            def make_h1(q):
                h1 = h1p.tile([HD, NK], bf16, tag="h1")
                nc.vector.tensor_scalar(
                    h1, gamma_b, beta[:, q:q + 1], 0.0,
                    AluOpType.add, AluOpType.max,
                )
                return h1

            h1t = {}
            chunks = {}

            # 70 of 256 drains go to DVE (it also carries all h1s), the
            # rest to ACT: balances DVE 481+1240f vs ACT 1135(1-f).
            NDVE = 70
            H1_AHEAD = 5
            for q in range(H1_AHEAD):
                h1t[q] = make_h1(q)

            for q in range(QPC):
                c, pos = q // CHQ, q % CHQ
                if pos == 0:
                    h2c = h2p.tile([HD, CHQ * NK], bf16, tag="h2c")
                    chunks[c] = h2c

                # next h1 first: its input (gamma) is always ready, so it
                # never blocks the DVE queue; the PE needs it soon.
                if q + H1_AHEAD < QPC:
                    h1t[q + H1_AHEAD] = make_h1(q + H1_AHEAD)

                p2 = ps2.tile([HD, NK], f32, tag="p2")
                h1 = h1t.pop(q)
                for kh in range(2):
                    nc.tensor.matmul(
                        p2[:, kh * KT:(kh + 1) * KT],
                        w2r,
                        h1[:, kh * KT:(kh + 1) * KT],
                        start=True,
                        stop=True,
                    )

                dst = chunks[c][:, pos * NK:(pos + 1) * NK]
                dve_drain = (q * NDVE) // QPC != ((q + 1) * NDVE) // QPC
                if dve_drain:
                    if with_b2:
                        nc.vector.tensor_scalar(
                            dst, p2, b2, 0.0, AluOpType.add, AluOpType.max,
                        )
                    else:
                        nc.vector.tensor_scalar_max(dst, p2, 0.0)
                else:
                    if with_b2:
                        nc.scalar.activation(dst, p2, Relu, bias=b2)
                    else:
                        nc.scalar.activation(dst, p2, Relu)

                # every chunk goes out as two 2MB halves (q,k fused into
                # one contiguous dim): the first half's store starts 8
                # queries early, keeping the DMA pipeline ~6us ahead and
                # shrinking the end-of-kernel tail to one half-transfer.
                if pos == CHQ // 2 - 1 or pos == CHQ - 1:
                    h = 0 if pos == CHQ // 2 - 1 else 1
                    hw = CHQ * NK // 2
                    tile_src = chunks.pop(c) if h == 1 else chunks[c]
                    half = bass.AP(
                        tensor=out_d.tensor,
                        offset=out_d.offset + c * CHQ * NK + h * hw,
                        ap=[[QPC * NK, HD], [1, hw]],
                    )
                    nc.sync.dma_start(half, tile_src[:, h * hw:(h + 1) * hw])

    nc.compile()
    return nc


def _get_nc(with_b2: bool):
    key = ("nc", with_b2)
    if key not in _CACHE:
        _CACHE[key] = _build_nc(with_b2)
    return _CACHE[key]


def make_in_maps(query_coords, key_coords, w1, b1, w2, b2):
    """Host-side shard prep: per-core gamma/beta + replicated weights."""
    qc = np.asarray(query_coords, np.float32)
    kc = np.asarray(key_coords, np.float32)
    w1 = np.asarray(w1, np.float32)
    b1 = np.asarray(b1, np.float32)
    w2 = np.asarray(w2, np.float32)
    b2 = np.asarray(b2, np.float32)

    import ml_dtypes

    with_b2 = bool(np.any(b2))
    w2c = np.ascontiguousarray(w2.astype(ml_dtypes.bfloat16))
    b2c = np.ascontiguousarray(b2.reshape(HD, 1))

    in_maps = []
    for c in range(NCORES):
        b = c // CPB
        q0 = (c % CPB) * QPC
        gamma = np.ascontiguousarray(
            (-(kc[b] @ w1).T).astype(ml_dtypes.bfloat16)         # (128, NK)
        )
        beta = np.ascontiguousarray(
            (qc[b, q0:q0 + QPC] @ w1).T + b1[:, None]            # (128, QPC)
        )
        m = {"gamma": gamma, "beta": beta, "w2": w2c}
        if with_b2:
            m["b2"] = b2c
        in_maps.append(m)
    return in_maps, with_b2


def assemble_output(results, w3, b3):
    """Host layer 3: gather per-core h2 [HD, QPC, NK] bf16, contract the
    128 hidden dims to 8 heads in f32, into (B, H, NQ, NK) f32."""
    w3 = np.asarray(w3, np.float32)
    b3 = np.asarray(b3, np.float32)
    out = np.empty((B, H, NQ, NK), np.float32)
    w3t = np.ascontiguousarray(w3.T)                             # (H, HD)
    for c in range(NCORES):
        b = c // CPB
        q0 = (c % CPB) * QPC
        h2 = results[c]["out"]
        # exact bf16 -> f32 without ml_dtypes' slow cast path
        h2f = (
            (h2.view(np.uint16).astype(np.uint32) << 16)
            .view(np.float32)
            .reshape(HD, QPC * NK)
        )
        out[b, :, q0:q0 + QPC, :] = (w3t @ h2f).reshape(H, QPC, NK)
    if np.any(b3):
        out += b3.reshape(1, H, 1, 1)
    return out


def kernel(**inputs):
    from concourse.bass_utils import run_bass_kernel_spmd

    in_maps, with_b2 = make_in_maps(
        inputs["query_coords"],
        inputs["key_coords"],
        inputs["w1"],
        inputs["b1"],
        inputs["w2"],
        inputs["b2"],
    )
    nc = _get_nc(with_b2)
    res = run_bass_kernel_spmd(nc, in_maps, list(range(NCORES)))
    return assemble_output(res.results, inputs["w3"], inputs["b3"])
